# revision 1
# baseline (speedup 1.0000x reference)
"""Multi-head self-attention (pre-LN, residual) Trainium2 Bass kernel.

Problem: B=4, S=2048, D=128, H=4, Dh=32, fp32 -> rel err ~1.2e-3.
Sharding: 8 cores = 4 batches x 2 query-halves (1024 queries/core).
Each core receives its batch's full x, row-shuffled by the host so that
(a) the core's query half occupies device positions 0..1023 (attention is
permutation-invariant over keys) and (b) each SBUF partition loads
consecutive DRAM rows (8KB-contiguous DMA chunks at full bandwidth).

Fully transposed dataflow ([feature, seq] layouts) so the softmax
reduction rides the PE and no giant P-matrix transpose is needed:
  xn0^T --W--> Q^T,K^T [hd, s] bf16;  V [s, hd] bf16
  S^T[k,q] = K^T.T @ Q^T     2+2 heads packed via PE row-tiling (K=32)
  P_A = exp(S^T - 8)         heads {0,2} on ACT (table exp, bf16 out)
  P_B = schraudolph(S^T - 8) heads {1,3} on DVE: ONE tensor_scalar
                             (x*SA+SB) with int16 convert-on-write whose
                             bits are bf16 exp (min-RMS corrected, ~2%)
  ctx^T[hd,q] = V.T @ P      4 heads packed via PE col-tiling (M=32)
  den[hd,q]   = 1.T @ P      col-tiled ones-matmul (per-head row blocks)
  out^T = Wo.T @ (ctx^T * recip_approx(den)) + (x^T + bias)
gamma/beta/all biases are folded into projection weights / per-partition
bias columns.  QKV/out projections run as float32r (tf32-like); scores
and P-side matmuls in bf16; all PSUM accumulation fp32.

Scheduling: LN/transpose/projection prep is emitted in 4-tile blocks
interleaved with the attention k-loop (blocks 2,3 inject into chunk 0);
scores+exps are emitted one ktile ahead of ctx/den so the in-order PE
stream always has runnable work while exps are in flight; softmax recip
uses the custom-DVE fast reciprocal (~51 ULP); dummy full-array matmul
bursts warm the PE HAM clock-gate (tile_position'd matmuls alone do not
hold it at 2.4 GHz).
"""

import sys

if "/opt/trn_rl_repo" not in sys.path:
    sys.path.insert(0, "/opt/trn_rl_repo")

import numpy as np

import concourse.bacc as bacc
import concourse.tile as tile
import concourse.mybir as mybir
from concourse.bass_utils import run_bass_kernel_spmd
from concourse.masks import make_identity

F32 = mybir.dt.float32
F32R = mybir.dt.float32r
BF16 = mybir.dt.bfloat16
I16 = mybir.dt.int16
AF = mybir.ActivationFunctionType
OP = mybir.AluOpType

B, S, D = 4, 2048, 128
H, DH = 4, 32
N_CORES = 8
QH = S // 2  # queries per core
NT = S // 128  # 16 s-tiles
NQT = QH // 128  # 8 q-tiles
CHUNK = 512
NCH = QH // CHUNK  # q-chunks per core
NKT = S // 128  # k-tiles
EPS = 1e-6
SHIFT = 8.0
ISQ = 1.0 / np.sqrt(np.float32(DH))
# Schraudolph bf16 exp: int16(x*SA + SB).bits == bf16(exp(x - SHIFT))
SA = float(128.0 / np.log(2.0))
SB = float(127.0 * 128.0 - 0.0579 * 128.0 - SHIFT * 128.0 / np.log(2.0))

GROUPS = ((0, 2), (1, 3))  # (A on ACT, B on DVE); same-parity heads share
# a ctxden bank so Wo row masks stay partition-aligned.

_compiled = None


def _build():
    nc = bacc.Bacc(
        "TRN2",
        target_bir_lowering=False,
        debug=False,
        enable_asserts=False,
        num_devices=N_CORES,
    )

    xkv_d = nc.dram_tensor("xkv", [S, D], F32, kind="ExternalInput").ap()
    wq_d = nc.dram_tensor("wq", [D, D], F32, kind="ExternalInput").ap()
    wk_d = nc.dram_tensor("wk", [D, D], F32, kind="ExternalInput").ap()
    wv_d = nc.dram_tensor("wv", [D, D], F32, kind="ExternalInput").ap()
    wo_d = nc.dram_tensor("wo", [D, D], F32, kind="ExternalInput").ap()
    # rows: gamma, beta, bq, bk, bv, bo
    vecs_d = nc.dram_tensor("vecs", [6, D], F32, kind="ExternalInput").ap()
    outT_d = nc.dram_tensor("outT", [D, QH], F32, kind="ExternalOutput").ap()

    with tile.TileContext(nc) as tc:
        consts = tc.alloc_tile_pool(name="consts", bufs=1)
        sbW = tc.alloc_tile_pool(name="sbW", bufs=1)
        sbBig = tc.alloc_tile_pool(name="sbBig", bufs=1)
        sbTmp = tc.alloc_tile_pool(name="sbTmp", bufs=3)

        ident = consts.tile([128, 128], F32)
        make_identity(nc, ident)
        nshift = consts.tile([128, 1], F32)
        nc.vector.memset(nshift, -SHIFT)
        epsc = consts.tile([128, 1], F32)
        nc.vector.memset(epsc, EPS)
        zeroc = consts.tile([128, 1], F32)
        nc.vector.memset(zeroc, 0.0)
        wsrc = consts.tile([128, 512], BF16)
        nc.vector.memset(wsrc, 0.5)
        wones = consts.tile([128, DH], BF16)
        nc.vector.memset(wones, 1.0)

        # ---- input DMAs ----
        wq_raw = sbW.tile([D, D], F32)
        wk_raw = sbW.tile([D, D], F32)
        wv_raw = sbW.tile([D, D], F32)
        wo_raw = sbW.tile([D, D], F32)
        nc.scalar.dma_start(out=wq_raw, in_=wq_d)
        nc.scalar.dma_start(out=wk_raw, in_=wk_d)
        nc.scalar.dma_start(out=wv_raw, in_=wv_d)
        nc.scalar.dma_start(out=wo_raw, in_=wo_d)
        smallT = sbW.tile([D, 6], F32)  # cols: gamma,beta,bq,bk,bv,bo
        nc.scalar.dma_start(out=smallT, in_=vecs_d.rearrange("v d -> d v"))

        xkv_sb = sbBig.tile([128, NT, 128], F32)
        xkv_r = xkv_d.rearrange("(p t) d -> p t d", t=NT)
        for c4 in range(4):
            nc.sync.dma_start(
                out=xkv_sb[:, c4 * 4 : (c4 + 1) * 4, :],
                in_=xkv_r[:, c4 * 4 : (c4 + 1) * 4, :],
            )

        ps_a = tc.alloc_tile_pool(name="ps_a", bufs=2, space="PSUM")

        # HAM warm-up chain (independent; fills PE during DVE/DMA setup)
        for _ in range(8):
            wps = ps_a.tile([128, 512], F32, name="wps", tag="a")
            nc.tensor.matmul(wps[0:DH, :], wones, wsrc, start=True, stop=True)

        # ---- fold gamma/beta/biases ----
        gam = smallT[:, 0:1]
        bet = smallT[:, 1:2]
        gq = sbW.tile([128, 1], F32)
        nc.vector.tensor_scalar_mul(gq, gam, float(ISQ))
        wq_f = sbW.tile([D, D], F32R)
        wk_f = sbW.tile([D, D], F32R)
        wv_f = sbW.tile([D, D], F32R)
        nc.vector.tensor_scalar_mul(wq_f, wq_raw, gq)
        nc.vector.tensor_scalar_mul(wk_f, wk_raw, gam)
        nc.vector.tensor_scalar_mul(wv_f, wv_raw, gam)

        wo_r = sbW.tile([D, D], F32R)
        nc.vector.tensor_copy(wo_r, wo_raw)
        bqe = sbW.tile([128, 1], F32)
        bke = sbW.tile([128, 1], F32)
        bve = sbW.tile([128, 1], F32)
        rbias = sbW.tile([128, 1], F32)
        t_ps = ps_a.tile([128, 1], F32, tag="a")
        nc.tensor.matmul(t_ps, wq_raw, bet, start=True, stop=True)
        nc.vector.tensor_scalar(
            bqe, t_ps, smallT[:, 2:3], float(ISQ), op0=OP.add, op1=OP.mult
        )
        t_ps = ps_a.tile([128, 1], F32, tag="a")
        nc.tensor.matmul(t_ps, wk_raw, bet, start=True, stop=True)
        nc.vector.tensor_scalar_add(bke, t_ps, smallT[:, 3:4])
        t_ps = ps_a.tile([128, 1], F32, tag="a")
        nc.tensor.matmul(t_ps, wv_raw, bet, start=True, stop=True)
        nc.vector.tensor_scalar_add(bve, t_ps, smallT[:, 4:5])
        t_ps = ps_a.tile([128, 1], F32, tag="a")
        nc.tensor.matmul(t_ps, wo_raw, bve, start=True, stop=True)
        nc.vector.tensor_scalar_add(rbias, t_ps, smallT[:, 5:6])


        # ---- LayerNorm + transposes + projections, pipelined with the
        # attention loop: prep block b covers s-tiles 4b..4b+3 (their LN,
        # transpose, K/Q projection chunk and V tiles); attention ktiles
        # 4b..4b+3 of chunk 0 only need blocks <= b, so emission interleaves
        # prep blocks with attention ktiles and the PE stream never waits on
        # the full setup.
        mv_all = sbBig.tile([128, NT, 2], F32)
        lnv = sbBig.tile([128, NT], F32)
        rs_all = sbBig.tile([128, NT], F32)
        bias2 = sbBig.tile([128, NT], F32)
        xn0_sb = sbBig.tile([128, NT, 128], F32)
        xkvT = sbBig.tile([128, S], F32R)  # xn0^T [d, s]
        kT = sbBig.tile([128, S], BF16)
        qT = sbBig.tile([128, QH], BF16)
        v_sb = sbBig.tile([128, NT, 128], BF16)
        residT = sbBig.tile([128, QH], F32)  # x^T + resid_bias (query half)

        def prep_block(b4):
            for t in range(b4 * 4, b4 * 4 + 4):
                stats = sbTmp.tile([128, 6], F32, tag="st")
                nc.vector.bn_stats(stats, xkv_sb[:, t, :])
                nc.vector.bn_aggr(mv_all[:, t, :], stats)
            sl4 = slice(b4 * 4, b4 * 4 + 4)
            # rs = sqrt(1/(var+eps)): reciprocal exact on DVE, Sqrt on ACT
            nc.vector.tensor_scalar_add(lnv[:, sl4], mv_all[:, sl4, 1], epsc)
            nc.vector.reciprocal(bias2[:, sl4], lnv[:, sl4])
            nc.scalar.activation(
                rs_all[:, sl4], bias2[:, sl4], AF.Sqrt, bias=zeroc, scale=1.0
            )
            for t in range(b4 * 4, b4 * 4 + 4):
                nc.vector.tensor_scalar(
                    xn0_sb[:, t, :],
                    xkv_sb[:, t, :],
                    mv_all[:, t, 0:1],
                    rs_all[:, t : t + 1],
                    op0=OP.subtract,
                    op1=OP.mult,
                )
                tp = ps_a.tile([128, 128], F32, tag="a")
                nc.tensor.transpose(tp, xn0_sb[:, t, :], ident)
                nc.scalar.copy(xkvT[:, t * 128 : (t + 1) * 128], tp)
            c = b4
            pp = ps_a.tile([128, CHUNK], F32, tag="a")
            nc.tensor.matmul(
                pp, wk_f, xkvT[:, c * CHUNK : (c + 1) * CHUNK], start=True, stop=True
            )
            nc.vector.tensor_scalar_add(kT[:, c * CHUNK : (c + 1) * CHUNK], pp, bke)
            if c < NCH:
                pp = ps_a.tile([128, CHUNK], F32, tag="a")
                nc.tensor.matmul(
                    pp, wq_f, xkvT[:, c * CHUNK : (c + 1) * CHUNK],
                    start=True, stop=True,
                )
                nc.vector.tensor_scalar_add(
                    qT[:, c * CHUNK : (c + 1) * CHUNK], pp, bqe
                )
            for t in range(b4 * 4, b4 * 4 + 4):
                pp = ps_a.tile([128, 128], F32, tag="a")
                nc.tensor.matmul(
                    pp, xkvT[:, t * 128 : (t + 1) * 128], wv_f, start=True, stop=True
                )
                nc.scalar.copy(v_sb[:, t, :], pp)

        def resid_block(ts_range):
            for t in ts_range:
                tp = ps_a.tile([128, 128], F32, tag="a")
                nc.tensor.transpose(tp, xkv_sb[:, t, :], ident)
                nc.vector.tensor_scalar_add(
                    residT[:, t * 128 : (t + 1) * 128], tp, rbias
                )

        # ---- attention (interleaved with prep blocks) ----
        ps_e = tc.alloc_tile_pool(name="ps_e", bufs=1, space="PSUM")
        pPool = tc.alloc_tile_pool(name="pPool", bufs=6)

        ctx_sb = sbBig.tile([128, NCH, CHUNK], F32)
        den_all = sbBig.tile([128, NCH, CHUNK], F32)
        ctx_ps = None
        den_ps = None

        def attn_scores(qc, kt):
            q0 = qc * CHUNK
            k0 = kt * 128
            p_sb = [None, None]
            for g, heads in enumerate(GROUPS):
                sp = ps_e.tile([128, 2 * CHUNK], F32, name=f"s{g}", tag="s", bufs=2)
                for i, h in enumerate(heads):
                    nc.tensor.matmul(
                        sp[:, i * CHUNK : (i + 1) * CHUNK],
                        kT[h * DH : (h + 1) * DH, k0 : k0 + 128],
                        qT[h * DH : (h + 1) * DH, q0 : q0 + CHUNK],
                        start=True,
                        stop=True,
                        tile_position=(h * DH, 0),
                    )
                if g == 0:
                    pA = pPool.tile([128, 2 * CHUNK], BF16, tag="p")
                    nc.scalar.activation(pA, sp, AF.Exp, bias=nshift, scale=1.0)
                    p_sb[0] = pA
                else:
                    pB = pPool.tile([128, 2 * CHUNK], I16, tag="p")
                    nc.vector.tensor_scalar(pB, sp, SA, SB, op0=OP.mult, op1=OP.add)
                    p_sb[1] = pB.bitcast(BF16)
            return p_sb

        def attn_ctxden(qc, kt, p_sb):
            for g, heads in enumerate(GROUPS):
                for i, h in enumerate(heads):
                    nc.tensor.matmul(
                        ctx_ps[h * DH : (h + 1) * DH, :],
                        v_sb[:, kt, h * DH : (h + 1) * DH],
                        p_sb[g][:, i * CHUNK : (i + 1) * CHUNK],
                        start=(kt == 0),
                        stop=(kt == NKT - 1),
                        tile_position=(0, h * DH),
                    )
            for g, heads in enumerate(GROUPS):
                for i, h in enumerate(heads):
                    nc.tensor.matmul(
                        den_ps[h * DH : (h + 1) * DH, :],
                        wones,
                        p_sb[g][:, i * CHUNK : (i + 1) * CHUNK],
                        start=(kt == 0),
                        stop=(kt == NKT - 1),
                        tile_position=(0, h * DH),
                    )

        prep_block(0)
        resid_block(range(0, 4))
        prep_block(1)
        resid_block(range(4, NQT))
        for _ in range(6):
            warm2 = ps_a.tile([128, CHUNK], F32, name="warm2", tag="a")
            nc.tensor.matmul(warm2[0:DH, :], wones, wsrc, start=True, stop=True)

        def chunk_tail(qc):
            q0 = qc * CHUNK
            ctxn = sbTmp.tile([128, CHUNK], F32R, tag="cn")
            nc.vector.tensor_mul(ctxn, ctx_sb[:, qc, :], den_all[:, qc, :])
            out_ps = ps_a.tile([128, CHUNK], F32, name="out_ps", tag="a")
            nc.tensor.matmul(out_ps, wo_r, ctxn, start=True, stop=True)
            fin = sbTmp.tile([128, CHUNK], F32, tag="fin")
            nc.vector.tensor_add(fin, out_ps, residT[:, q0 : q0 + CHUNK])
            nc.sync.dma_start(out=outT_d[:, q0 : q0 + CHUNK], in_=fin)

        # chunk 0 interleaved with remaining prep; scores/exp emitted one
        # ktile ahead of ctx/den so the in-order PE stream always has
        # runnable work while the exps of the previous ktile are in flight.
        ctx_ps = ps_e.tile([128, CHUNK], F32, name="ctx0", tag="ctx")
        den_ps = ps_e.tile([128, CHUNK], F32, name="den0", tag="den")
        pending = attn_scores(0, 0)
        for kt in range(NKT):
            if kt == 3:
                prep_block(2)
            elif kt == 7:
                prep_block(3)
            nxt = attn_scores(0, kt + 1) if kt + 1 < NKT else None
            attn_ctxden(0, kt, pending)
            pending = nxt
        nc.vector.tensor_copy(ctx_sb[:, 0, :], ctx_ps)
        nc.vector.reciprocal_approx_fast(den_all[:, 0, :], den_ps)

        # chunk 1
        ctx_ps = ps_e.tile([128, CHUNK], F32, name="ctx1", tag="ctx")
        den_ps = ps_e.tile([128, CHUNK], F32, name="den1", tag="den")
        for _ in range(4):
            warm3 = ps_a.tile([128, CHUNK], F32, name="warm3", tag="a")
            nc.tensor.matmul(warm3[0:DH, :], wones, wsrc, start=True, stop=True)
        pending = attn_scores(1, 0)
        for kt in range(NKT):
            nxt = attn_scores(1, kt + 1) if kt + 1 < NKT else None
            attn_ctxden(1, kt, pending)
            pending = nxt
        nc.vector.tensor_copy(ctx_sb[:, 1, :], ctx_ps)
        nc.vector.reciprocal_approx_fast(den_all[:, 1, :], den_ps)
        chunk_tail(0)
        chunk_tail(1)

        pPool.release()
        ps_e.release()
        ps_a.release()
        sbTmp.release()
        sbBig.release()
        sbW.release()
        consts.release()

    nc.compile()
    return nc


def _get_compiled():
    global _compiled
    if _compiled is None:
        _compiled = _build()
    return _compiled


# device position j <- host row (j%128)*16 + j//128
_DEV2HOST = (np.arange(S) % 128) * NT + np.arange(S) // 128
_HOSTPERM = np.empty(S, dtype=np.int64)
_HOSTPERM[_DEV2HOST] = np.arange(S)


def kernel(x, Wq, bq, Wk, bk, Wv, bv, gamma, beta, Wo, bo):
    x = np.asarray(x, dtype=np.float32)
    vecs = np.stack(
        [np.asarray(a, dtype=np.float32) for a in (gamma, beta, bq, bk, bv, bo)]
    )
    wq = np.ascontiguousarray(np.asarray(Wq, dtype=np.float32))
    wk = np.ascontiguousarray(np.asarray(Wk, dtype=np.float32))
    wv = np.ascontiguousarray(np.asarray(Wv, dtype=np.float32))
    wo = np.ascontiguousarray(np.asarray(Wo, dtype=np.float32))

    nc = _get_compiled()

    in_maps = []
    for c in range(N_CORES):
        b, half = c // 2, c % 2
        off = half * QH
        xroll = np.roll(x[b], -off, axis=0)
        xin = np.ascontiguousarray(xroll[_HOSTPERM])
        in_maps.append(
            {"xkv": xin, "wq": wq, "wk": wk, "wv": wv, "wo": wo, "vecs": vecs}
        )

    res = run_bass_kernel_spmd(nc, in_maps, core_ids=list(range(N_CORES)), trace=False)

    out = np.empty((B, S, D), dtype=np.float32)
    for c in range(N_CORES):
        b, half = c // 2, c % 2
        off = half * QH
        out[b, off : off + QH, :] = res.results[c]["outT"].T
    return out



# revision 4
# speedup vs baseline: 1.2540x; 1.2540x over previous
"""Multi-head self-attention (pre-LN, residual) Trainium2 Bass kernel.

Problem: B=4, S=2048, D=128, H=4, Dh=32, fp32 -> rel err ~1.2e-3.
Sharding: 8 cores = 4 batches x 2 query-halves (1024 queries/core).
Each core receives its batch's full x, row-shuffled by the host so that
(a) the core's query half occupies device positions 0..1023 (attention is
permutation-invariant over keys) and (b) each SBUF partition loads
consecutive DRAM rows (8KB-contiguous DMA chunks at full bandwidth).

Fully transposed dataflow ([feature, seq] layouts) so the softmax
reduction rides the PE and no giant P-matrix transpose is needed:
  xn0^T --W--> Q^T,K^T [hd, s] bf16;  V [s, hd] bf16
  S^T[k,q] = K^T.T @ Q^T     2+2 heads packed via PE row-tiling (K=32)
  P_A = exp(S^T - 8)         heads {0,2} on ACT (table exp, bf16 out)
  P_B = schraudolph(S^T - 8) heads {1,3} on DVE: ONE tensor_scalar
                             (x*SA+SB) with int16 convert-on-write whose
                             bits are bf16 exp (min-RMS corrected, ~2%)
  ctx^T[hd,q] = V.T @ P      4 heads packed via PE col-tiling (M=32)
  den[hd,q]   = 1.T @ P      col-tiled ones-matmul (per-head row blocks)
  out^T = Wo.T @ (ctx^T * recip_approx(den)) + (x^T + bias)
gamma/beta/all biases are folded into projection weights / per-partition
bias columns.  QKV/out projections run as float32r (tf32-like); scores
and P-side matmuls in bf16; all PSUM accumulation fp32.

PSUM layout (8 banks): one shared tag-"s" ring of 3 x [128,1024] slots
(6 banks) holds scores tiles AND all prep/warm/out-proj psum tiles, plus
1 bank each for the ctx / den accumulators.  The 3-deep ring is the key
scheduling device: with only 2 slots the group-A scores of ktile j+1
must wait for exp_A(j) to release its bank, serializing
[exp -> scores -> exp] on one engine (measured 1737ns/ktile); with 3
slots the WAR partner alternates groups (sA(j+1) waits exp_B(j-1),
sB(j+1) waits exp_A(j)), the chain spreads over ACT+DVE in a 3-ktile
cycle, and the PE becomes the pacer.

LN rs = rsqrt(var+eps) via Quake bit-trick + 2 Newton steps on the DVE
(4.7e-6 rel err) so the ACT never loads the sqrt table set: the exp
table is loaded once (pre-warmed by a dummy exp during the input DMA)
and never swapped.  Prep transposes/V-projections batch 4 tiles into
one psum bank -> single 512-wide ACT copy; K/Q bias adds ride the ACT
(Identity+bias) instead of the DVE.  Chunk-0's tail (softmax divide,
out-projection, residual add, output DMA) is injected into chunk-1's
k-loop so it overlaps the second half of attention.
"""

import sys

if "/opt/trn_rl_repo" not in sys.path:
    sys.path.insert(0, "/opt/trn_rl_repo")

import numpy as np

import concourse.bacc as bacc
import concourse.tile as tile
import concourse.mybir as mybir
from concourse.bass_utils import run_bass_kernel_spmd
from concourse.masks import make_identity

F32 = mybir.dt.float32
F32R = mybir.dt.float32r
BF16 = mybir.dt.bfloat16
I16 = mybir.dt.int16
I32 = mybir.dt.int32
AF = mybir.ActivationFunctionType
OP = mybir.AluOpType

B, S, D = 4, 2048, 128
H, DH = 4, 32
N_CORES = 8
QH = S // 2  # queries per core
NT = S // 128  # 16 s-tiles
NQT = QH // 128  # 8 q-tiles
CHUNK = 512
NCH = QH // CHUNK  # q-chunks per core
NKT = S // 128  # k-tiles
EPS = 1e-6
SHIFT = 8.0
ISQ = 1.0 / np.sqrt(np.float32(DH))
# Schraudolph bf16 exp: int16(x*SA + SB).bits == bf16(exp(x - SHIFT))
SA = float(128.0 / np.log(2.0))
SB = float(127.0 * 128.0 - 0.0579 * 128.0 - SHIFT * 128.0 / np.log(2.0))
QMAGIC = 0x5F3759E0  # 0x5f3759df + 1 (C - x == ~x + C + 1)

GROUPS = ((0, 2), (1, 3))  # (A on ACT, B on DVE); same-parity heads share
# a ctxden bank so Wo row masks stay partition-aligned.

_compiled = None


def _build():
    nc = bacc.Bacc(
        "TRN2",
        target_bir_lowering=False,
        debug=False,
        enable_asserts=False,
        num_devices=N_CORES,
    )

    xkv_d = nc.dram_tensor("xkv", [S, D], F32, kind="ExternalInput").ap()
    wq_d = nc.dram_tensor("wq", [D, D], F32, kind="ExternalInput").ap()
    wk_d = nc.dram_tensor("wk", [D, D], F32, kind="ExternalInput").ap()
    wv_d = nc.dram_tensor("wv", [D, D], F32, kind="ExternalInput").ap()
    wo_d = nc.dram_tensor("wo", [D, D], F32, kind="ExternalInput").ap()
    # rows: gamma, beta, bq, bk, bv, bo
    vecs_d = nc.dram_tensor("vecs", [6, D], F32, kind="ExternalInput").ap()
    outT_d = nc.dram_tensor("outT", [D, QH], F32, kind="ExternalOutput").ap()

    with tile.TileContext(nc) as tc:
        consts = tc.alloc_tile_pool(name="consts", bufs=1)
        sbW = tc.alloc_tile_pool(name="sbW", bufs=1)
        sbBig = tc.alloc_tile_pool(name="sbBig", bufs=1)
        sbTmp = tc.alloc_tile_pool(name="sbTmp", bufs=3)

        ident = consts.tile([128, 128], F32)
        make_identity(nc, ident)
        nshift = consts.tile([128, 1], F32)
        nc.vector.memset(nshift, -SHIFT)
        epsc = consts.tile([128, 1], F32)
        nc.vector.memset(epsc, EPS)
        wsrc = consts.tile([128, 512], BF16)
        nc.vector.memset(wsrc, 0.5)
        wones = consts.tile([128, DH], BF16)
        nc.vector.memset(wones, 1.0)
        c1i = consts.tile([128, 1], I32)
        nc.vector.memset(c1i, 1)
        cmaski = consts.tile([128, 1], I32)
        nc.vector.memset(cmaski, -1)
        cmagici = consts.tile([128, 4], I32)
        nc.vector.memset(cmagici, QMAGIC)
        # pre-warm the ACT exp table set during the input DMAs
        expwarm = consts.tile([128, 1], F32)
        nc.scalar.activation(expwarm, epsc, AF.Exp, bias=0.0, scale=1.0)

        # ---- input DMAs ----
        xkv_sb = sbBig.tile([128, NT, 128], F32)
        xkv_r = xkv_d.rearrange("(p t) d -> p t d", t=NT)
        for c4 in range(4):
            nc.sync.dma_start(
                out=xkv_sb[:, c4 * 4 : (c4 + 1) * 4, :],
                in_=xkv_r[:, c4 * 4 : (c4 + 1) * 4, :],
            )
        wq_raw = sbW.tile([D, D], F32)
        wk_raw = sbW.tile([D, D], F32)
        wv_raw = sbW.tile([D, D], F32)
        wo_raw = sbW.tile([D, D], F32)
        nc.scalar.dma_start(out=wq_raw, in_=wq_d)
        nc.scalar.dma_start(out=wk_raw, in_=wk_d)
        nc.scalar.dma_start(out=wv_raw, in_=wv_d)
        nc.scalar.dma_start(out=wo_raw, in_=wo_d)
        smallT = sbW.tile([D, 6], F32)  # cols: gamma,beta,bq,bk,bv,bo
        nc.scalar.dma_start(out=smallT, in_=vecs_d.rearrange("v d -> d v"))

        ps = tc.alloc_tile_pool(name="ps", bufs=3, space="PSUM")

        # HAM warm-up chain (independent; fills PE during DVE/DMA setup)
        for _ in range(8):
            wps = ps.tile([128, 512], F32, name="wps", tag="s")
            nc.tensor.matmul(wps[0:DH, :], wones, wsrc, start=True, stop=True)

        # ---- fold gamma/beta/biases ----
        gam = smallT[:, 0:1]
        bet = smallT[:, 1:2]
        gq = sbW.tile([128, 1], F32)
        nc.vector.tensor_scalar_mul(gq, gam, float(ISQ))
        wq_f = sbW.tile([D, D], F32R)
        wk_f = sbW.tile([D, D], F32R)
        wv_f = sbW.tile([D, D], F32R)
        nc.vector.tensor_scalar_mul(wq_f, wq_raw, gq)
        nc.vector.tensor_scalar_mul(wk_f, wk_raw, gam)
        nc.vector.tensor_scalar_mul(wv_f, wv_raw, gam)

        wo_r = sbW.tile([D, D], F32R)
        nc.vector.tensor_copy(wo_r, wo_raw)
        bqe = sbW.tile([128, 1], F32)
        bke = sbW.tile([128, 1], F32)
        bve = sbW.tile([128, 1], F32)
        rbias = sbW.tile([128, 1], F32)
        t_ps = ps.tile([128, 1], F32, name="t_ps", tag="s")
        nc.tensor.matmul(t_ps, wq_raw, bet, start=True, stop=True)
        nc.vector.tensor_scalar(
            bqe, t_ps, smallT[:, 2:3], float(ISQ), op0=OP.add, op1=OP.mult
        )
        t_ps = ps.tile([128, 1], F32, name="t_ps", tag="s")
        nc.tensor.matmul(t_ps, wk_raw, bet, start=True, stop=True)
        nc.vector.tensor_scalar_add(bke, t_ps, smallT[:, 3:4])
        t_ps = ps.tile([128, 1], F32, name="t_ps", tag="s")
        nc.tensor.matmul(t_ps, wv_raw, bet, start=True, stop=True)
        nc.vector.tensor_scalar_add(bve, t_ps, smallT[:, 4:5])
        t_ps = ps.tile([128, 1], F32, name="t_ps", tag="s")
        nc.tensor.matmul(t_ps, wo_raw, bve, start=True, stop=True)
        nc.vector.tensor_scalar_add(rbias, t_ps, smallT[:, 5:6])

        # ---- LayerNorm + transposes + projections, pipelined with the
        # attention loop: prep block b covers s-tiles 4b..4b+3; attention
        # ktiles 4b..4b+3 of chunk 0 only need blocks <= b, so blocks 2,3
        # are injected into chunk-0's k-loop at kt 3 and 7.
        mv_all = sbBig.tile([128, NT, 2], F32)
        lnv = sbBig.tile([128, NT], F32)
        lnvi = lnv.bitcast(I32)
        rs_all = sbBig.tile([128, NT], F32)
        qy = sbBig.tile([128, NT], F32)
        qyi = qy.bitcast(I32)
        qs = sbBig.tile([128, NT], F32)
        qsi = qs.bitcast(I32)
        xn0_sb = sbBig.tile([128, NT, 128], F32)
        xkvT = sbBig.tile([128, S], F32R)  # xn0^T [d, s]
        kT = sbBig.tile([128, S], BF16)
        qT = sbBig.tile([128, QH], BF16)
        v_sb = sbBig.tile([128, NT, 128], BF16)
        residT = sbBig.tile([128, QH], F32)  # x^T + resid_bias (query half)

        def prep_block(b4):
            for t in range(b4 * 4, b4 * 4 + 4):
                stats = sbTmp.tile([128, 6], F32, tag="st")
                nc.vector.bn_stats(stats, xkv_sb[:, t, :])
                nc.vector.bn_aggr(mv_all[:, t, :], stats)
            sl4 = slice(b4 * 4, b4 * 4 + 4)
            nc.vector.tensor_scalar_add(lnv[:, sl4], mv_all[:, sl4, 1], epsc)
            # rs = rsqrt(var+eps): Quake bit-trick + 2 Newton steps, all DVE
            nc.vector.tensor_scalar(
                qsi[:, sl4], lnvi[:, sl4], c1i, cmaski,
                op0=OP.logical_shift_right, op1=OP.bitwise_xor,
            )
            nc.vector.tensor_add(qyi[:, sl4], qsi[:, sl4], cmagici)
            for last in (False, True):
                nc.vector.tensor_mul(qs[:, sl4], lnv[:, sl4], qy[:, sl4])
                nc.vector.tensor_mul(qs[:, sl4], qs[:, sl4], qy[:, sl4])
                nc.vector.tensor_scalar(
                    qs[:, sl4], qs[:, sl4], -0.5, 1.5, op0=OP.mult, op1=OP.add
                )
                dst = rs_all if last else qy
                nc.vector.tensor_mul(dst[:, sl4], qy[:, sl4], qs[:, sl4])
            for t in range(b4 * 4, b4 * 4 + 4):
                nc.vector.tensor_scalar(
                    xn0_sb[:, t, :],
                    xkv_sb[:, t, :],
                    mv_all[:, t, 0:1],
                    rs_all[:, t : t + 1],
                    op0=OP.subtract,
                    op1=OP.mult,
                )
            tp4 = ps.tile([128, 512], F32, name="tp4", tag="s")
            for i, t in enumerate(range(b4 * 4, b4 * 4 + 4)):
                nc.tensor.transpose(
                    tp4[:, i * 128 : (i + 1) * 128], xn0_sb[:, t, :], ident
                )
            nc.scalar.copy(xkvT[:, b4 * 512 : (b4 + 1) * 512], tp4)
            c = b4
            ppk = ps.tile([128, CHUNK], F32, name="ppk", tag="s")
            nc.tensor.matmul(
                ppk, wk_f, xkvT[:, c * CHUNK : (c + 1) * CHUNK], start=True, stop=True
            )
            nc.scalar.add(kT[:, c * CHUNK : (c + 1) * CHUNK], ppk, bke)
            if c < NCH:
                ppq = ps.tile([128, CHUNK], F32, name="ppq", tag="s")
                nc.tensor.matmul(
                    ppq, wq_f, xkvT[:, c * CHUNK : (c + 1) * CHUNK],
                    start=True, stop=True,
                )
                nc.scalar.add(qT[:, c * CHUNK : (c + 1) * CHUNK], ppq, bqe)
            ppv = ps.tile([128, 512], F32, name="ppv", tag="s")
            for i, t in enumerate(range(b4 * 4, b4 * 4 + 4)):
                nc.tensor.matmul(
                    ppv[:, i * 128 : (i + 1) * 128],
                    xkvT[:, t * 128 : (t + 1) * 128],
                    wv_f,
                    start=True,
                    stop=True,
                )
            nc.scalar.copy(v_sb[:, b4 * 4 : (b4 + 1) * 4, :], ppv)

        def resid_block(half):
            tp4r = ps.tile([128, 512], F32, name="tp4r", tag="s")
            for i, t in enumerate(range(half * 4, half * 4 + 4)):
                nc.tensor.transpose(
                    tp4r[:, i * 128 : (i + 1) * 128], xkv_sb[:, t, :], ident
                )
            nc.vector.tensor_scalar_add(
                residT[:, half * 512 : (half + 1) * 512], tp4r, rbias
            )

        # ---- attention ----
        pPool = tc.alloc_tile_pool(name="pPool", bufs=8)

        den_rec = sbBig.tile([128, NCH, CHUNK], F32)
        ctx_ps = None
        den_ps = None

        def attn_scores(qc, kt):
            q0 = qc * CHUNK
            k0 = kt * 128
            p_sb = [None, None]
            for g, heads in enumerate(GROUPS):
                sp = ps.tile([128, 2 * CHUNK], F32, name=f"s{g}", tag="s")
                for i, h in enumerate(heads):
                    nc.tensor.matmul(
                        sp[:, i * CHUNK : (i + 1) * CHUNK],
                        kT[h * DH : (h + 1) * DH, k0 : k0 + 128],
                        qT[h * DH : (h + 1) * DH, q0 : q0 + CHUNK],
                        start=True,
                        stop=True,
                        tile_position=(h * DH, 0),
                    )
                if g == 0:
                    pA = pPool.tile([128, 2 * CHUNK], BF16, tag="p")
                    nc.scalar.activation(pA, sp, AF.Exp, bias=nshift, scale=1.0)
                    p_sb[0] = pA
                else:
                    pB = pPool.tile([128, 2 * CHUNK], I16, tag="p")
                    nc.vector.tensor_scalar(pB, sp, SA, SB, op0=OP.mult, op1=OP.add)
                    p_sb[1] = pB.bitcast(BF16)
            return p_sb

        def attn_ctxden(qc, kt, p_sb):
            for g, heads in enumerate(GROUPS):
                for i, h in enumerate(heads):
                    nc.tensor.matmul(
                        ctx_ps[h * DH : (h + 1) * DH, :],
                        v_sb[:, kt, h * DH : (h + 1) * DH],
                        p_sb[g][:, i * CHUNK : (i + 1) * CHUNK],
                        start=(kt == 0),
                        stop=(kt == NKT - 1),
                        tile_position=(0, h * DH),
                    )
            for g, heads in enumerate(GROUPS):
                for i, h in enumerate(heads):
                    nc.tensor.matmul(
                        den_ps[h * DH : (h + 1) * DH, :],
                        wones,
                        p_sb[g][:, i * CHUNK : (i + 1) * CHUNK],
                        start=(kt == 0),
                        stop=(kt == NKT - 1),
                        tile_position=(0, h * DH),
                    )

        def chunk_tail(qc, ctx_psum):
            q0 = qc * CHUNK
            ctxn = sbTmp.tile([128, CHUNK], F32R, tag="cn")
            nc.vector.tensor_mul(ctxn, ctx_psum, den_rec[:, qc, :])
            out_ps = ps.tile([128, CHUNK], F32, name=f"outps{qc}", tag="s")
            nc.tensor.matmul(out_ps, wo_r, ctxn, start=True, stop=True)
            fin = sbTmp.tile([128, CHUNK], F32, tag="fin")
            nc.vector.tensor_add(fin, out_ps, residT[:, q0 : q0 + CHUNK])
            nc.sync.dma_start(out=outT_d[:, q0 : q0 + CHUNK], in_=fin)

        prep_block(0)
        resid_block(0)
        prep_block(1)
        resid_block(1)

        # chunk 0 (prep blocks 2,3 injected at kt 3 and 7); scores emitted
        # one ktile ahead of ctx/den so the PE always has runnable work.
        ctx_ps = ps.tile([128, CHUNK], F32, name="ctx0", tag="ctx", bufs=1)
        den_ps = ps.tile([128, CHUNK], F32, name="den0", tag="den", bufs=1)
        pending = attn_scores(0, 0)
        for kt in range(NKT):
            if kt == 3:
                prep_block(2)
            elif kt == 7:
                prep_block(3)
            nxt = attn_scores(0, kt + 1) if kt + 1 < NKT else None
            attn_ctxden(0, kt, pending)
            pending = nxt
        nc.vector.reciprocal_approx_fast(den_rec[:, 0, :], den_ps)
        ctx0_ps = ctx_ps

        # chunk 1; chunk-0's tail is injected at kt 2 so its divide /
        # out-projection / residual / DMA overlap chunk-1's attention.
        ctx_ps = ps.tile([128, CHUNK], F32, name="ctx1", tag="ctx", bufs=1)
        den_ps = ps.tile([128, CHUNK], F32, name="den1", tag="den", bufs=1)
        pending = attn_scores(1, 0)
        for kt in range(NKT):
            if kt == 2:
                chunk_tail(0, ctx0_ps)
            nxt = attn_scores(1, kt + 1) if kt + 1 < NKT else None
            attn_ctxden(1, kt, pending)
            pending = nxt
        nc.vector.reciprocal_approx_fast(den_rec[:, 1, :], den_ps)
        chunk_tail(1, ctx_ps)

        pPool.release()
        ps.release()
        sbTmp.release()
        sbBig.release()
        sbW.release()
        consts.release()

    nc.compile()
    return nc


def _get_compiled():
    global _compiled
    if _compiled is None:
        _compiled = _build()
    return _compiled


# device position j <- host row (j%128)*16 + j//128
_DEV2HOST = (np.arange(S) % 128) * NT + np.arange(S) // 128
_HOSTPERM = np.empty(S, dtype=np.int64)
_HOSTPERM[_DEV2HOST] = np.arange(S)


def kernel(x, Wq, bq, Wk, bk, Wv, bv, gamma, beta, Wo, bo):
    x = np.asarray(x, dtype=np.float32)
    vecs = np.stack(
        [np.asarray(a, dtype=np.float32) for a in (gamma, beta, bq, bk, bv, bo)]
    )
    wq = np.ascontiguousarray(np.asarray(Wq, dtype=np.float32))
    wk = np.ascontiguousarray(np.asarray(Wk, dtype=np.float32))
    wv = np.ascontiguousarray(np.asarray(Wv, dtype=np.float32))
    wo = np.ascontiguousarray(np.asarray(Wo, dtype=np.float32))

    nc = _get_compiled()

    in_maps = []
    for c in range(N_CORES):
        b, half = c // 2, c % 2
        off = half * QH
        xroll = np.roll(x[b], -off, axis=0)
        xin = np.ascontiguousarray(xroll[_HOSTPERM])
        in_maps.append(
            {"xkv": xin, "wq": wq, "wk": wk, "wv": wv, "wo": wo, "vecs": vecs}
        )

    res = run_bass_kernel_spmd(nc, in_maps, core_ids=list(range(N_CORES)), trace=False)

    out = np.empty((B, S, D), dtype=np.float32)
    for c in range(N_CORES):
        b, half = c // 2, c % 2
        off = half * QH
        out[b, off : off + QH, :] = res.results[c]["outT"].T
    return out


# revision 17
# speedup vs baseline: 1.3108x; 1.0453x over previous
"""Multi-head self-attention (pre-LN, residual) Trainium2 Bass kernel.

Problem: B=4, S=2048, D=128, H=4, Dh=32, fp32 -> rel err ~1.2e-3.
Sharding: 8 cores = 4 batches x 2 query-halves (1024 queries/core).
Each core receives its batch's full x, row-shuffled by the host so that
(a) the core's query half occupies device positions 0..1023 (attention is
permutation-invariant over keys) and (b) each SBUF partition loads
consecutive DRAM rows (8KB-contiguous DMA chunks at full bandwidth).

Fully transposed dataflow ([feature, seq] layouts) so the softmax
reduction rides the PE and no giant P-matrix transpose is needed:
  xn0^T --W--> Q^T,K^T [hd, s] bf16;  V [s, hd] bf16
  S^T[k,q] = K^T.T @ Q^T     2+2 heads packed via PE row-tiling (K=32)
  P_A = exp(S^T - 8)         heads {0,2} on ACT (table exp, bf16 out)
  P_B = schraudolph(S^T - 8) heads {1,3} on DVE: ONE tensor_scalar
                             (x*SA+SB) with int16 convert-on-write whose
                             bits are bf16 exp (min-RMS corrected, ~2%)
  ctx^T[hd,q] = V.T @ P      4 heads packed via PE col-tiling (M=32)
  den[hd,q]   = 1.T @ P      col-tiled ones-matmul (per-head row blocks)
  out^T = Wo.T @ (ctx^T * recip_approx(den)) + (x^T + bias)
gamma/beta/all biases are folded into projection weights / per-partition
bias columns.  QKV/out projections run as float32r (tf32-like); scores
and P-side matmuls in bf16; all PSUM accumulation fp32.

PSUM layout (8 banks): one shared tag-"s" ring of 3 x [128,1024] slots
(6 banks) holds scores tiles AND all prep/warm/out-proj psum tiles, plus
1 bank each for the ctx / den accumulators.  The 3-deep ring is the key
scheduling device: with only 2 slots the group-A scores of ktile j+1
must wait for exp_A(j) to release its bank, serializing
[exp -> scores -> exp] on one engine (measured 1737ns/ktile); with 3
slots the WAR partner alternates groups (sA(j+1) waits exp_B(j-1),
sB(j+1) waits exp_A(j)), the chain spreads over ACT+DVE in a 3-ktile
cycle, and the PE becomes the pacer.

LN rs = rsqrt(var+eps) via Quake bit-trick + 2 Newton steps on the DVE
(4.7e-6 rel err) so the ACT never loads the sqrt table set: the exp
table is loaded once (pre-warmed by a dummy exp during the input DMA)
and never swapped.  Prep transposes/V-projections batch 4 tiles into
one psum bank -> single 512-wide ACT copy; K/Q bias adds ride the ACT
(Identity+bias) instead of the DVE.  Chunk-0's tail (softmax divide,
out-projection, residual add, output DMA) is injected into chunk-1's
k-loop so it overlaps the second half of attention.
"""

import sys

if "/opt/trn_rl_repo" not in sys.path:
    sys.path.insert(0, "/opt/trn_rl_repo")

import ml_dtypes
import numpy as np

import concourse.bacc as bacc
import concourse.tile as tile
import concourse.mybir as mybir
from concourse.bass_utils import run_bass_kernel_spmd
from concourse.masks import make_identity

F32 = mybir.dt.float32
F32R = mybir.dt.float32r
BF16 = mybir.dt.bfloat16
I16 = mybir.dt.int16
I32 = mybir.dt.int32
AF = mybir.ActivationFunctionType
OP = mybir.AluOpType

B, S, D = 4, 2048, 128
H, DH = 4, 32
N_CORES = 8
QH = S // 2  # queries per core
NT = S // 128  # 16 s-tiles
NQT = QH // 128  # 8 q-tiles
CHUNK = 512
NCH = QH // CHUNK  # q-chunks per core
NKT = S // 128  # k-tiles
EPS = 1e-6
SHIFT = 8.0
ISQ = 1.0 / np.sqrt(np.float32(DH))
# Schraudolph bf16 exp: int16(x*SA + SB).bits == bf16(exp(x - SHIFT))
SA = float(128.0 / np.log(2.0))
SB = float(127.0 * 128.0 - 0.0579 * 128.0 - SHIFT * 128.0 / np.log(2.0))
QMAGIC = 0x5F3759E0  # 0x5f3759df + 1 (C - x == ~x + C + 1)

GROUPS = ((0, 2), (1, 3))  # (A on ACT, B on DVE); same-parity heads share
# a ctxden bank so Wo row masks stay partition-aligned.

_compiled = None


def _build():
    nc = bacc.Bacc(
        "TRN2",
        target_bir_lowering=False,
        debug=False,
        enable_asserts=False,
        num_devices=N_CORES,
    )

    xkv_d = nc.dram_tensor("xkv", [S, D], BF16, kind="ExternalInput").ap()
    wq_d = nc.dram_tensor("wq", [D, D], F32, kind="ExternalInput").ap()
    wk_d = nc.dram_tensor("wk", [D, D], F32, kind="ExternalInput").ap()
    wv_d = nc.dram_tensor("wv", [D, D], F32, kind="ExternalInput").ap()
    wo_d = nc.dram_tensor("wo", [D, D], F32, kind="ExternalInput").ap()
    # rows: gamma, beta, bq, bk, bv, bo
    vecs_d = nc.dram_tensor("vecs", [6, D], F32, kind="ExternalInput").ap()
    outT_d = nc.dram_tensor("outT", [D, QH], F32, kind="ExternalOutput").ap()

    with tile.TileContext(nc) as tc:
        consts = tc.alloc_tile_pool(name="consts", bufs=1)
        sbW = tc.alloc_tile_pool(name="sbW", bufs=1)
        sbBig = tc.alloc_tile_pool(name="sbBig", bufs=1)
        sbTmp = tc.alloc_tile_pool(name="sbTmp", bufs=3)

        ident = consts.tile([128, 128], F32)
        make_identity(nc, ident)
        ident_bf = consts.tile([128, 128], BF16)
        nc.vector.tensor_copy(ident_bf, ident)
        nshift = consts.tile([128, 1], F32)
        nc.vector.memset(nshift, -SHIFT)
        epsc = consts.tile([128, 1], F32)
        nc.vector.memset(epsc, EPS)
        wsrc = consts.tile([128, 512], BF16)
        nc.vector.memset(wsrc, 0.5)
        wones = consts.tile([128, DH], BF16)
        nc.vector.memset(wones, 1.0)
        c1i = consts.tile([128, 1], I32)
        nc.vector.memset(c1i, 1)
        cmaski = consts.tile([128, 1], I32)
        nc.vector.memset(cmaski, -1)
        cmagici = consts.tile([128, 4], I32)
        nc.vector.memset(cmagici, QMAGIC)
        # pre-warm the ACT exp table set during the input DMAs
        expwarm = consts.tile([128, 1], F32)
        nc.scalar.activation(expwarm, epsc, AF.Exp, bias=0.0, scale=1.0)

        # ---- input DMAs (small tensors first so weight folds start early) ----
        wq_raw = sbW.tile([D, D], F32)
        wk_raw = sbW.tile([D, D], F32)
        wv_raw = sbW.tile([D, D], F32)
        wo_raw = sbW.tile([D, D], F32)
        nc.scalar.dma_start(out=wq_raw, in_=wq_d)
        nc.scalar.dma_start(out=wk_raw, in_=wk_d)
        nc.scalar.dma_start(out=wv_raw, in_=wv_d)
        nc.scalar.dma_start(out=wo_raw, in_=wo_d)
        smallT = sbW.tile([D, 6], F32)  # cols: gamma,beta,bq,bk,bv,bo
        nc.scalar.dma_start(out=smallT, in_=vecs_d.rearrange("v d -> d v"))
        xkv_sb = sbBig.tile([128, NT, 128], BF16)
        xkv_r = xkv_d.rearrange("(p t) d -> p t d", t=NT)
        for c4 in range(4):
            nc.sync.dma_start(
                out=xkv_sb[:, c4 * 4 : (c4 + 1) * 4, :],
                in_=xkv_r[:, c4 * 4 : (c4 + 1) * 4, :],
            )

        ps = tc.alloc_tile_pool(name="ps", bufs=3, space="PSUM")

        # HAM warm-up chain (independent; fills PE during DVE/DMA setup)
        for _ in range(8):
            wps = ps.tile([128, 512], F32, name="wps", tag="s")
            nc.tensor.matmul(wps[0:DH, :], wones, wsrc, start=True, stop=True)

        # ---- fold gamma/beta/biases ----
        gam = smallT[:, 0:1]
        bet = smallT[:, 1:2]
        gq = sbW.tile([128, 1], F32)
        nc.vector.tensor_scalar_mul(gq, gam, float(ISQ))
        wq_f = sbW.tile([D, D], BF16)
        wk_f = sbW.tile([D, D], BF16)
        wv_f = sbW.tile([D, D], BF16)
        nc.vector.tensor_scalar_mul(wq_f, wq_raw, gq)
        nc.vector.tensor_scalar_mul(wk_f, wk_raw, gam)
        nc.vector.tensor_scalar_mul(wv_f, wv_raw, gam)

        wo_r = sbW.tile([D, D], BF16)
        nc.vector.tensor_copy(wo_r, wo_raw)
        bqe = sbW.tile([128, 1], F32)
        bke = sbW.tile([128, 1], F32)
        bve = sbW.tile([128, 1], F32)
        rbias = sbW.tile([128, 1], F32)
        t_ps = ps.tile([128, 1], F32, name="t_ps", tag="s")
        nc.tensor.matmul(t_ps, wq_raw, bet, start=True, stop=True)
        nc.vector.tensor_scalar(
            bqe, t_ps, smallT[:, 2:3], float(ISQ), op0=OP.add, op1=OP.mult
        )
        t_ps = ps.tile([128, 1], F32, name="t_ps", tag="s")
        nc.tensor.matmul(t_ps, wk_raw, bet, start=True, stop=True)
        nc.vector.tensor_scalar_add(bke, t_ps, smallT[:, 3:4])
        t_ps = ps.tile([128, 1], F32, name="t_ps", tag="s")
        nc.tensor.matmul(t_ps, wv_raw, bet, start=True, stop=True)
        nc.vector.tensor_scalar_add(bve, t_ps, smallT[:, 4:5])
        t_ps = ps.tile([128, 1], F32, name="t_ps", tag="s")
        nc.tensor.matmul(t_ps, wo_raw, bve, start=True, stop=True)
        nc.vector.tensor_scalar_add(rbias, t_ps, smallT[:, 5:6])

        # extra warm burst to bridge the gap between setup and prep matmuls
        for _ in range(4):
            wps2 = ps.tile([128, 512], F32, name="wps2", tag="s")
            nc.tensor.matmul(wps2[0:DH, :], wones, wsrc, start=True, stop=True)

        # ---- LayerNorm + transposes + projections, pipelined with the
        # attention loop: prep block b covers s-tiles 4b..4b+3; attention
        # ktiles 4b..4b+3 of chunk 0 only need blocks <= b, so blocks 2,3
        # are injected into chunk-0's k-loop at kt 3 and 7.
        mv_all = sbBig.tile([128, NT, 2], F32)
        lnv = sbBig.tile([128, NT], F32)
        lnvi = lnv.bitcast(I32)
        rs_all = sbBig.tile([128, NT], F32)
        qy = sbBig.tile([128, NT], F32)
        qyi = qy.bitcast(I32)
        qs = sbBig.tile([128, NT], F32)
        qsi = qs.bitcast(I32)
        xn0_sb = sbBig.tile([128, NT, 128], BF16)
        xkvT = sbBig.tile([128, S], BF16)  # xn0^T [d, s]
        kT = sbBig.tile([128, S], BF16)
        qT = sbBig.tile([128, QH], BF16)
        v_sb = sbBig.tile([128, NT, 128], BF16)
        residT = sbBig.tile([128, QH], F32)  # x^T + resid_bias (query half)

        def prep_block(b4):
            for t in range(b4 * 4, b4 * 4 + 4):
                stats = sbTmp.tile([128, 6], F32, tag="st")
                nc.vector.bn_stats(stats, xkv_sb[:, t, :])
                nc.vector.bn_aggr(mv_all[:, t, :], stats)
            sl4 = slice(b4 * 4, b4 * 4 + 4)
            nc.vector.tensor_scalar_add(lnv[:, sl4], mv_all[:, sl4, 1], epsc)
            # rs = rsqrt(var+eps): Quake bit-trick + 2 Newton steps, all DVE
            nc.vector.tensor_scalar(
                qsi[:, sl4], lnvi[:, sl4], c1i, cmaski,
                op0=OP.logical_shift_right, op1=OP.bitwise_xor,
            )
            nc.vector.tensor_add(qyi[:, sl4], qsi[:, sl4], cmagici)
            for last in (False, True):
                nc.vector.tensor_mul(qs[:, sl4], lnv[:, sl4], qy[:, sl4])
                nc.vector.tensor_mul(qs[:, sl4], qs[:, sl4], qy[:, sl4])
                nc.vector.tensor_scalar(
                    qs[:, sl4], qs[:, sl4], -0.5, 1.5, op0=OP.mult, op1=OP.add
                )
                dst = rs_all if last else qy
                nc.vector.tensor_mul(dst[:, sl4], qy[:, sl4], qs[:, sl4])
            for t in range(b4 * 4, b4 * 4 + 4):
                nc.vector.tensor_scalar(
                    xn0_sb[:, t, :],
                    xkv_sb[:, t, :],
                    mv_all[:, t, 0:1],
                    rs_all[:, t : t + 1],
                    op0=OP.subtract,
                    op1=OP.mult,
                )
            tp4 = ps.tile([128, 512], BF16, name="tp4", tag="s")
            for i, t in enumerate(range(b4 * 4, b4 * 4 + 4)):
                nc.tensor.transpose(
                    tp4[:, i * 128 : (i + 1) * 128], xn0_sb[:, t, :], ident_bf
                )
            nc.scalar.copy(xkvT[:, b4 * 512 : (b4 + 1) * 512], tp4)
            c = b4
            ppk = ps.tile([128, CHUNK], F32, name="ppk", tag="s")
            nc.tensor.matmul(
                ppk, wk_f, xkvT[:, c * CHUNK : (c + 1) * CHUNK], start=True, stop=True
            )
            nc.scalar.add(kT[:, c * CHUNK : (c + 1) * CHUNK], ppk, bke)
            if c < NCH:
                ppq = ps.tile([128, CHUNK], F32, name="ppq", tag="s")
                nc.tensor.matmul(
                    ppq, wq_f, xkvT[:, c * CHUNK : (c + 1) * CHUNK],
                    start=True, stop=True,
                )
                nc.scalar.add(qT[:, c * CHUNK : (c + 1) * CHUNK], ppq, bqe)
            ppv = ps.tile([128, 512], F32, name="ppv", tag="s")
            for i, t in enumerate(range(b4 * 4, b4 * 4 + 4)):
                nc.tensor.matmul(
                    ppv[:, i * 128 : (i + 1) * 128],
                    xkvT[:, t * 128 : (t + 1) * 128],
                    wv_f,
                    start=True,
                    stop=True,
                )
            nc.scalar.copy(v_sb[:, b4 * 4 : (b4 + 1) * 4, :], ppv)

        def resid_block(half):
            tp4r = ps.tile([128, 512], BF16, name="tp4r", tag="s")
            for i, t in enumerate(range(half * 4, half * 4 + 4)):
                nc.tensor.transpose(
                    tp4r[:, i * 128 : (i + 1) * 128], xkv_sb[:, t, :], ident_bf
                )
            nc.vector.tensor_scalar_add(
                residT[:, half * 512 : (half + 1) * 512], tp4r, rbias
            )

        # ---- attention ----
        pPool = tc.alloc_tile_pool(name="pPool", bufs=8)

        den_rec = sbBig.tile([128, NCH, CHUNK], F32)
        ctx_ps = None
        den_ps = None

        def attn_scores(qc, kt):
            q0 = qc * CHUNK
            k0 = kt * 128
            p_sb = [None, None]
            for g, heads in enumerate(GROUPS):
                sp = ps.tile([128, 2 * CHUNK], F32, name=f"s{g}", tag="s")
                for i, h in enumerate(heads):
                    nc.tensor.matmul(
                        sp[:, i * CHUNK : (i + 1) * CHUNK],
                        kT[h * DH : (h + 1) * DH, k0 : k0 + 128],
                        qT[h * DH : (h + 1) * DH, q0 : q0 + CHUNK],
                        start=True,
                        stop=True,
                        tile_position=(h * DH, 0),
                    )
                if g == 0:
                    pA = pPool.tile([128, 2 * CHUNK], BF16, tag="p")
                    nc.scalar.activation(pA, sp, AF.Exp, bias=nshift, scale=1.0)
                    p_sb[0] = pA
                else:
                    pB = pPool.tile([128, 2 * CHUNK], I16, tag="p")
                    nc.vector.tensor_scalar(pB, sp, SA, SB, op0=OP.mult, op1=OP.add)
                    p_sb[1] = pB.bitcast(BF16)
            return p_sb

        def attn_ctxden(qc, kt, p_sb):
            for g, heads in enumerate(GROUPS):
                for i, h in enumerate(heads):
                    nc.tensor.matmul(
                        ctx_ps[h * DH : (h + 1) * DH, :],
                        v_sb[:, kt, h * DH : (h + 1) * DH],
                        p_sb[g][:, i * CHUNK : (i + 1) * CHUNK],
                        start=(kt == 0),
                        stop=(kt == NKT - 1),
                        tile_position=(0, h * DH),
                    )
            for g, heads in enumerate(GROUPS):
                for i, h in enumerate(heads):
                    nc.tensor.matmul(
                        den_ps[h * DH : (h + 1) * DH, :],
                        wones,
                        p_sb[g][:, i * CHUNK : (i + 1) * CHUNK],
                        start=(kt == 0),
                        stop=(kt == NKT - 1),
                        tile_position=(0, h * DH),
                    )

        def chunk_tail(qc, ctx_psum):
            q0 = qc * CHUNK
            ctxn = sbTmp.tile([128, CHUNK], BF16, tag="cn")
            nc.vector.tensor_mul(ctxn, ctx_psum, den_rec[:, qc, :])
            out_ps = ps.tile([128, CHUNK], F32, name=f"outps{qc}", tag="s")
            nc.tensor.matmul(out_ps, wo_r, ctxn, start=True, stop=True)
            fin = sbTmp.tile([128, CHUNK], F32, tag="fin")
            nc.vector.tensor_add(fin, out_ps, residT[:, q0 : q0 + CHUNK])
            nc.sync.dma_start(out=outT_d[:, q0 : q0 + CHUNK], in_=fin)

        prep_block(0)
        resid_block(0)

        # chunk 0 (prep blocks 2,3 injected at kt 3 and 7); scores emitted
        # TWO ktiles ahead of ctx/den — the 3-slot sp ring then keeps the
        # PE fed and decouples each scores pair from the exp two ktiles
        # back. kt0/kt1 scores are emitted before prep_block(1) so their
        # ring slots don't chain behind block-1's DMA-gated prep.
        ctx_ps = ps.tile([128, CHUNK], F32, name="ctx0", tag="ctx", bufs=1)
        den_ps = ps.tile([128, CHUNK], F32, name="den0", tag="den", bufs=1)
        pend = [attn_scores(0, 0), attn_scores(0, 1)]
        prep_block(1)
        resid_block(1)
        for kt in range(NKT):
            if kt == 3:
                prep_block(2)
            elif kt == 7:
                prep_block(3)
            if kt + 2 < NKT:
                pend.append(attn_scores(0, kt + 2))
            attn_ctxden(0, kt, pend.pop(0))
        nc.vector.reciprocal_approx_fast(den_rec[:, 0, :], den_ps)
        ctx0_ps = ctx_ps

        # chunk 1; chunk-0's tail is injected at kt 2 so its divide /
        # out-projection / residual / DMA overlap chunk-1's attention.
        ctx_ps = ps.tile([128, CHUNK], F32, name="ctx1", tag="ctx", bufs=1)
        den_ps = ps.tile([128, CHUNK], F32, name="den1", tag="den", bufs=1)
        pend = [attn_scores(1, 0), attn_scores(1, 1)]
        for kt in range(NKT):
            if kt == 2:
                chunk_tail(0, ctx0_ps)
            if kt + 2 < NKT:
                pend.append(attn_scores(1, kt + 2))
            attn_ctxden(1, kt, pend.pop(0))
        nc.vector.reciprocal_approx_fast(den_rec[:, 1, :], den_ps)
        chunk_tail(1, ctx_ps)

        pPool.release()
        ps.release()
        sbTmp.release()
        sbBig.release()
        sbW.release()
        consts.release()

    nc.compile()
    return nc


def _get_compiled():
    global _compiled
    if _compiled is None:
        _compiled = _build()
    return _compiled


# device position j <- host row (j%128)*16 + j//128
_DEV2HOST = (np.arange(S) % 128) * NT + np.arange(S) // 128
_HOSTPERM = np.empty(S, dtype=np.int64)
_HOSTPERM[_DEV2HOST] = np.arange(S)


def kernel(x, Wq, bq, Wk, bk, Wv, bv, gamma, beta, Wo, bo):
    x = np.asarray(x, dtype=np.float32)
    vecs = np.stack(
        [np.asarray(a, dtype=np.float32) for a in (gamma, beta, bq, bk, bv, bo)]
    )
    wq = np.ascontiguousarray(np.asarray(Wq, dtype=np.float32))
    wk = np.ascontiguousarray(np.asarray(Wk, dtype=np.float32))
    wv = np.ascontiguousarray(np.asarray(Wv, dtype=np.float32))
    wo = np.ascontiguousarray(np.asarray(Wo, dtype=np.float32))

    nc = _get_compiled()

    in_maps = []
    for c in range(N_CORES):
        b, half = c // 2, c % 2
        off = half * QH
        xroll = np.roll(x[b], -off, axis=0)
        xin = np.ascontiguousarray(xroll[_HOSTPERM].astype(ml_dtypes.bfloat16))
        in_maps.append(
            {"xkv": xin, "wq": wq, "wk": wk, "wv": wv, "wo": wo, "vecs": vecs}
        )

    res = run_bass_kernel_spmd(nc, in_maps, core_ids=list(range(N_CORES)), trace=False)

    out = np.empty((B, S, D), dtype=np.float32)
    for c in range(N_CORES):
        b, half = c // 2, c % 2
        off = half * QH
        out[b, off : off + QH, :] = res.results[c]["outT"].T
    return out


# revision 22
# speedup vs baseline: 1.3290x; 1.0139x over previous
"""Multi-head self-attention (pre-LN, residual) Trainium2 Bass kernel.

Problem: B=4, S=2048, D=128, H=4, Dh=32, fp32 -> rel err ~1.2e-3.
Sharding: 8 cores = 4 batches x 2 query-halves (1024 queries/core).
Each core receives its batch's full x, row-shuffled by the host so that
(a) the core's query half occupies device positions 0..1023 (attention is
permutation-invariant over keys) and (b) each SBUF partition loads
consecutive DRAM rows (8KB-contiguous DMA chunks at full bandwidth).

Fully transposed dataflow ([feature, seq] layouts) so the softmax
reduction rides the PE and no giant P-matrix transpose is needed:
  xn0^T --W--> Q^T,K^T [hd, s] bf16;  V [s, hd] bf16
  S^T[k,q] = K^T.T @ Q^T     2+2 heads packed via PE row-tiling (K=32)
  P_A = exp(S^T - 8)         heads {0,2} on ACT (table exp, bf16 out)
  P_B = schraudolph(S^T - 8) heads {1,3} on DVE: ONE tensor_scalar
                             (x*SA+SB) with int16 convert-on-write whose
                             bits are bf16 exp (min-RMS corrected, ~2%)
  ctx^T[hd,q] = V.T @ P      4 heads packed via PE col-tiling (M=32)
  den[hd,q]   = 1.T @ P      col-tiled ones-matmul (per-head row blocks)
  out^T = Wo.T @ (ctx^T * recip_approx(den)) + (x^T + bias)
gamma/beta/all biases are folded into projection weights / per-partition
bias columns.  QKV/out projections run as float32r (tf32-like); scores
and P-side matmuls in bf16; all PSUM accumulation fp32.

PSUM layout (8 banks): one shared tag-"s" ring of 3 x [128,1024] slots
(6 banks) holds scores tiles AND all prep/warm/out-proj psum tiles, plus
1 bank each for the ctx / den accumulators.  The 3-deep ring is the key
scheduling device: with only 2 slots the group-A scores of ktile j+1
must wait for exp_A(j) to release its bank, serializing
[exp -> scores -> exp] on one engine (measured 1737ns/ktile); with 3
slots the WAR partner alternates groups (sA(j+1) waits exp_B(j-1),
sB(j+1) waits exp_A(j)), the chain spreads over ACT+DVE in a 3-ktile
cycle, and the PE becomes the pacer.

LN rs = rsqrt(var+eps) via Quake bit-trick + 2 Newton steps on the DVE
(4.7e-6 rel err) so the ACT never loads the sqrt table set: the exp
table is loaded once (pre-warmed by a dummy exp during the input DMA)
and never swapped.  Prep transposes/V-projections batch 4 tiles into
one psum bank -> single 512-wide ACT copy; K/Q bias adds ride the ACT
(Identity+bias) instead of the DVE.  Chunk-0's tail (softmax divide,
out-projection, residual add, output DMA) is injected into chunk-1's
k-loop so it overlaps the second half of attention.
"""

import sys

if "/opt/trn_rl_repo" not in sys.path:
    sys.path.insert(0, "/opt/trn_rl_repo")

import ml_dtypes
import numpy as np

import concourse.bacc as bacc
import concourse.tile as tile
import concourse.mybir as mybir
from concourse.bass_utils import run_bass_kernel_spmd
from concourse.masks import make_identity

F32 = mybir.dt.float32
F32R = mybir.dt.float32r
BF16 = mybir.dt.bfloat16
I16 = mybir.dt.int16
I32 = mybir.dt.int32
AF = mybir.ActivationFunctionType
OP = mybir.AluOpType

B, S, D = 4, 2048, 128
H, DH = 4, 32
N_CORES = 8
QH = S // 2  # queries per core
NT = S // 128  # 16 s-tiles
NQT = QH // 128  # 8 q-tiles
CHUNK = 512
NCH = QH // CHUNK  # q-chunks per core
NKT = S // 128  # k-tiles
EPS = 1e-6
SHIFT = 8.0
ISQ = 1.0 / np.sqrt(np.float32(DH))
# Schraudolph bf16 exp: int16(x*SA + SB).bits == bf16(exp(x - SHIFT))
SA = float(128.0 / np.log(2.0))
SB = float(127.0 * 128.0 - 0.0579 * 128.0 - SHIFT * 128.0 / np.log(2.0))
QMAGIC = 0x5F3759E0  # 0x5f3759df + 1 (C - x == ~x + C + 1)

GROUPS = ((0, 2), (1, 3))  # (A on ACT, B on DVE); same-parity heads share
# a ctxden bank so Wo row masks stay partition-aligned.

_compiled = None


def _build():
    nc = bacc.Bacc(
        "TRN2",
        target_bir_lowering=False,
        debug=False,
        enable_asserts=False,
        num_devices=N_CORES,
    )

    xkv_d = nc.dram_tensor("xkv", [S, D], BF16, kind="ExternalInput").ap()
    wq_d = nc.dram_tensor("wq", [D, D], F32, kind="ExternalInput").ap()
    wk_d = nc.dram_tensor("wk", [D, D], F32, kind="ExternalInput").ap()
    wv_d = nc.dram_tensor("wv", [D, D], F32, kind="ExternalInput").ap()
    wo_d = nc.dram_tensor("wo", [D, D], F32, kind="ExternalInput").ap()
    # rows: gamma, beta, bq, bk, bv, bo
    vecs_d = nc.dram_tensor("vecs", [6, D], F32, kind="ExternalInput").ap()
    outT_d = nc.dram_tensor("outT", [D, QH], F32, kind="ExternalOutput").ap()

    with tile.TileContext(nc) as tc:
        consts = tc.alloc_tile_pool(name="consts", bufs=1)
        sbW = tc.alloc_tile_pool(name="sbW", bufs=1)
        sbBig = tc.alloc_tile_pool(name="sbBig", bufs=1)
        sbTmp = tc.alloc_tile_pool(name="sbTmp", bufs=3)

        ident = consts.tile([128, 128], F32)
        make_identity(nc, ident)
        ident_bf = consts.tile([128, 128], BF16)
        nc.vector.tensor_copy(ident_bf, ident)
        nshift = consts.tile([128, 1], F32)
        nc.vector.memset(nshift, -SHIFT)
        epsc = consts.tile([128, 1], F32)
        nc.vector.memset(epsc, EPS)
        wsrc = consts.tile([128, 512], BF16)
        nc.vector.memset(wsrc, 0.5)
        wones = consts.tile([128, DH], BF16)
        nc.vector.memset(wones, 1.0)
        c1i = consts.tile([128, 1], I32)
        nc.vector.memset(c1i, 1)
        cmaski = consts.tile([128, 1], I32)
        nc.vector.memset(cmaski, -1)
        cmagici = consts.tile([128, 4], I32)
        nc.vector.memset(cmagici, QMAGIC)
        # pre-warm the ACT exp table set during the input DMAs
        expwarm = consts.tile([128, 1], F32)
        nc.scalar.activation(expwarm, epsc, AF.Exp, bias=0.0, scale=1.0)

        # ---- input DMAs (small tensors first so weight folds start early;
        # vecs loaded in natural [6,128] layout — 6 big descriptors instead
        # of 768 4-byte strided ones — and transposed on the PE) ----
        wq_raw = sbW.tile([D, D], F32)
        wk_raw = sbW.tile([D, D], F32)
        wv_raw = sbW.tile([D, D], F32)
        wo_raw = sbW.tile([D, D], F32)
        vecs6 = sbW.tile([6, D], F32)
        nc.sync.dma_start(out=vecs6, in_=vecs_d)
        nc.sync.dma_start(out=wq_raw, in_=wq_d)
        nc.sync.dma_start(out=wk_raw, in_=wk_d)
        nc.sync.dma_start(out=wv_raw, in_=wv_d)
        nc.sync.dma_start(out=wo_raw, in_=wo_d)
        smallT = sbW.tile([D, 6], F32)  # cols: gamma,beta,bq,bk,bv,bo
        xkv_sb = sbBig.tile([128, NT, 128], BF16)
        xkv_r = xkv_d.rearrange("(p t) d -> p t d", t=NT)
        for c4 in range(4):
            nc.sync.dma_start(
                out=xkv_sb[:, c4 * 4 : (c4 + 1) * 4, :],
                in_=xkv_r[:, c4 * 4 : (c4 + 1) * 4, :],
            )

        ps = tc.alloc_tile_pool(name="ps", bufs=3, space="PSUM")

        # HAM warm-up chain (independent; fills PE during DVE/DMA setup)
        for _ in range(8):
            wps = ps.tile([128, 512], F32, name="wps", tag="s")
            nc.tensor.matmul(wps[0:DH, :], wones, wsrc, start=True, stop=True)

        # transpose vecs [6,128] -> smallT [128,6] on the PE
        tpv = ps.tile([128, 6], F32, name="tpv", tag="s")
        nc.tensor.transpose(tpv, vecs6, ident[0:6, 0:6])
        nc.scalar.copy(smallT, tpv)

        # ---- fold gamma/beta/biases ----
        gam = smallT[:, 0:1]
        bet = smallT[:, 1:2]
        gq = sbW.tile([128, 1], F32)
        nc.vector.tensor_scalar_mul(gq, gam, float(ISQ))
        wq_f = sbW.tile([D, D], BF16)
        wk_f = sbW.tile([D, D], BF16)
        wv_f = sbW.tile([D, D], BF16)
        nc.vector.tensor_scalar_mul(wq_f, wq_raw, gq)
        nc.vector.tensor_scalar_mul(wk_f, wk_raw, gam)
        nc.vector.tensor_scalar_mul(wv_f, wv_raw, gam)

        wo_r = sbW.tile([D, D], BF16)
        nc.vector.tensor_copy(wo_r, wo_raw)
        bqe = sbW.tile([128, 1], F32)
        bke = sbW.tile([128, 1], F32)
        bve = sbW.tile([128, 1], F32)
        rbias = sbW.tile([128, 1], F32)
        t_ps = ps.tile([128, 1], F32, name="t_ps", tag="s")
        nc.tensor.matmul(t_ps, wq_raw, bet, start=True, stop=True)
        nc.vector.tensor_scalar(
            bqe, t_ps, smallT[:, 2:3], float(ISQ), op0=OP.add, op1=OP.mult
        )
        t_ps = ps.tile([128, 1], F32, name="t_ps", tag="s")
        nc.tensor.matmul(t_ps, wk_raw, bet, start=True, stop=True)
        nc.vector.tensor_scalar_add(bke, t_ps, smallT[:, 3:4])
        t_ps = ps.tile([128, 1], F32, name="t_ps", tag="s")
        nc.tensor.matmul(t_ps, wv_raw, bet, start=True, stop=True)
        nc.vector.tensor_scalar_add(bve, t_ps, smallT[:, 4:5])
        t_ps = ps.tile([128, 1], F32, name="t_ps", tag="s")
        nc.tensor.matmul(t_ps, wo_raw, bve, start=True, stop=True)
        nc.vector.tensor_scalar_add(rbias, t_ps, smallT[:, 5:6])

        # warm burst to bridge the PE idle between setup and prep matmuls
        # (prep transposes wait for DMA-gated LN stats on the DVE)
        for _ in range(6):
            wps2 = ps.tile([128, 512], F32, name="wps2", tag="s")
            nc.tensor.matmul(wps2[0:DH, :], wones, wsrc, start=True, stop=True)

        # ---- LayerNorm + transposes + projections, pipelined with the
        # attention loop: prep block b covers s-tiles 4b..4b+3; attention
        # ktiles 4b..4b+3 of chunk 0 only need blocks <= b, so blocks 2,3
        # are injected into chunk-0's k-loop at kt 3 and 7.
        mv_all = sbBig.tile([128, NT, 2], F32)
        lnv = sbBig.tile([128, NT], F32)
        lnvi = lnv.bitcast(I32)
        rs_all = sbBig.tile([128, NT], F32)
        qy = sbBig.tile([128, NT], F32)
        qyi = qy.bitcast(I32)
        qs = sbBig.tile([128, NT], F32)
        qsi = qs.bitcast(I32)
        xn0_sb = sbBig.tile([128, NT, 128], BF16)
        xkvT = sbBig.tile([128, S], BF16)  # xn0^T [d, s]
        kT = sbBig.tile([128, S], BF16)
        qT = sbBig.tile([128, QH], BF16)
        v_sb = sbBig.tile([128, NT, 128], BF16)
        residT = sbBig.tile([128, QH], F32)  # x^T + resid_bias (query half)

        def prep_stats(ts_list):
            for t in ts_list:
                stats = sbTmp.tile([128, 6], F32, tag="st")
                nc.vector.bn_stats(stats, xkv_sb[:, t, :])
                nc.vector.bn_aggr(mv_all[:, t, :], stats)

        def prep_rsxn(b4):
            sl4 = slice(b4 * 4, b4 * 4 + 4)
            nc.vector.tensor_scalar_add(lnv[:, sl4], mv_all[:, sl4, 1], epsc)
            # rs = rsqrt(var+eps): Quake bit-trick + 2 Newton steps, all DVE
            nc.vector.tensor_scalar(
                qsi[:, sl4], lnvi[:, sl4], c1i, cmaski,
                op0=OP.logical_shift_right, op1=OP.bitwise_xor,
            )
            nc.vector.tensor_add(qyi[:, sl4], qsi[:, sl4], cmagici)
            for last in (False, True):
                nc.vector.tensor_mul(qs[:, sl4], lnv[:, sl4], qy[:, sl4])
                nc.vector.tensor_mul(qs[:, sl4], qs[:, sl4], qy[:, sl4])
                nc.vector.tensor_scalar(
                    qs[:, sl4], qs[:, sl4], -0.5, 1.5, op0=OP.mult, op1=OP.add
                )
                dst = rs_all if last else qy
                nc.vector.tensor_mul(dst[:, sl4], qy[:, sl4], qs[:, sl4])
            for t in range(b4 * 4, b4 * 4 + 4):
                nc.vector.tensor_scalar(
                    xn0_sb[:, t, :],
                    xkv_sb[:, t, :],
                    mv_all[:, t, 0:1],
                    rs_all[:, t : t + 1],
                    op0=OP.subtract,
                    op1=OP.mult,
                )

        def prep_mm_a(b4):
            tp4 = ps.tile([128, 512], BF16, name="tp4", tag="s")
            for i, t in enumerate(range(b4 * 4, b4 * 4 + 4)):
                nc.tensor.transpose(
                    tp4[:, i * 128 : (i + 1) * 128], xn0_sb[:, t, :], ident_bf
                )
            nc.scalar.copy(xkvT[:, b4 * 512 : (b4 + 1) * 512], tp4)

        def prep_mm_b(b4):
            c = b4
            # NOTE: K-bias is skipped — a per-hd-dim constant added to K
            # contributes the same value to every score in a softmax column
            # (sum_hd bke[hd]*qT[hd,q] is k-independent), so softmax is
            # exactly invariant to it.
            ppk = ps.tile([128, CHUNK], F32, name="ppk", tag="s")
            nc.tensor.matmul(
                ppk, wk_f, xkvT[:, c * CHUNK : (c + 1) * CHUNK], start=True, stop=True
            )
            nc.scalar.copy(kT[:, c * CHUNK : (c + 1) * CHUNK], ppk)
            if c < NCH:
                ppq = ps.tile([128, CHUNK], F32, name="ppq", tag="s")
                nc.tensor.matmul(
                    ppq, wq_f, xkvT[:, c * CHUNK : (c + 1) * CHUNK],
                    start=True, stop=True,
                )
                nc.scalar.add(qT[:, c * CHUNK : (c + 1) * CHUNK], ppq, bqe)
            ppv = ps.tile([128, 512], F32, name="ppv", tag="s")
            for i, t in enumerate(range(b4 * 4, b4 * 4 + 4)):
                nc.tensor.matmul(
                    ppv[:, i * 128 : (i + 1) * 128],
                    xkvT[:, t * 128 : (t + 1) * 128],
                    wv_f,
                    start=True,
                    stop=True,
                )
            nc.scalar.copy(v_sb[:, b4 * 4 : (b4 + 1) * 4, :], ppv)

        def prep_block(b4):
            prep_stats(range(b4 * 4, b4 * 4 + 4))
            prep_rsxn(b4)
            prep_mm_a(b4)
            prep_mm_b(b4)

        def resid_block(half):
            tp4r = ps.tile([128, 512], BF16, name="tp4r", tag="s")
            for i, t in enumerate(range(half * 4, half * 4 + 4)):
                nc.tensor.transpose(
                    tp4r[:, i * 128 : (i + 1) * 128], xkv_sb[:, t, :], ident_bf
                )
            nc.vector.tensor_scalar_add(
                residT[:, half * 512 : (half + 1) * 512], tp4r, rbias
            )

        # ---- attention ----
        pPool = tc.alloc_tile_pool(name="pPool", bufs=8)

        den_rec = sbBig.tile([128, NCH, CHUNK], F32)
        ctx_ps = None
        den_ps = None

        def attn_scores(qc, kt):
            q0 = qc * CHUNK
            k0 = kt * 128
            p_sb = [None, None]
            for g, heads in enumerate(GROUPS):
                sp = ps.tile([128, 2 * CHUNK], F32, name=f"s{g}", tag="s")
                for i, h in enumerate(heads):
                    nc.tensor.matmul(
                        sp[:, i * CHUNK : (i + 1) * CHUNK],
                        kT[h * DH : (h + 1) * DH, k0 : k0 + 128],
                        qT[h * DH : (h + 1) * DH, q0 : q0 + CHUNK],
                        start=True,
                        stop=True,
                        tile_position=(h * DH, 0),
                    )
                if g == 0:
                    pA = pPool.tile([128, 2 * CHUNK], BF16, tag="p")
                    nc.scalar.activation(pA, sp, AF.Exp, bias=nshift, scale=1.0)
                    p_sb[0] = pA
                else:
                    pB = pPool.tile([128, 2 * CHUNK], I16, tag="p")
                    nc.vector.tensor_scalar(pB, sp, SA, SB, op0=OP.mult, op1=OP.add)
                    p_sb[1] = pB.bitcast(BF16)
            return p_sb

        def attn_ctxden(qc, kt, p_sb):
            for g, heads in enumerate(GROUPS):
                for i, h in enumerate(heads):
                    nc.tensor.matmul(
                        ctx_ps[h * DH : (h + 1) * DH, :],
                        v_sb[:, kt, h * DH : (h + 1) * DH],
                        p_sb[g][:, i * CHUNK : (i + 1) * CHUNK],
                        start=(kt == 0),
                        stop=(kt == NKT - 1),
                        tile_position=(0, h * DH),
                    )
            for g, heads in enumerate(GROUPS):
                for i, h in enumerate(heads):
                    nc.tensor.matmul(
                        den_ps[h * DH : (h + 1) * DH, :],
                        wones,
                        p_sb[g][:, i * CHUNK : (i + 1) * CHUNK],
                        start=(kt == 0),
                        stop=(kt == NKT - 1),
                        tile_position=(0, h * DH),
                    )

        def chunk_tail(qc, ctx_psum):
            q0 = qc * CHUNK
            ctxn = sbTmp.tile([128, CHUNK], BF16, tag="cn")
            nc.vector.tensor_mul(ctxn, ctx_psum, den_rec[:, qc, :])
            out_ps = ps.tile([128, CHUNK], F32, name=f"outps{qc}", tag="s")
            nc.tensor.matmul(out_ps, wo_r, ctxn, start=True, stop=True)
            fin = sbTmp.tile([128, CHUNK], F32, tag="fin")
            nc.vector.tensor_add(fin, out_ps, residT[:, q0 : q0 + CHUNK])
            nc.sync.dma_start(out=outT_d[:, q0 : q0 + CHUNK], in_=fin)

        prep_block(0)
        resid_block(0)

        # chunk 0 (prep blocks 2,3 injected at kt 3 and 7); scores emitted
        # TWO ktiles ahead of ctx/den — the 3-slot sp ring then keeps the
        # PE fed and decouples each scores pair from the exp two ktiles
        # back. kt0/kt1 scores are emitted before prep_block(1) so their
        # ring slots don't chain behind block-1's DMA-gated prep.
        ctx_ps = ps.tile([128, CHUNK], F32, name="ctx0", tag="ctx", bufs=1)
        den_ps = ps.tile([128, CHUNK], F32, name="den0", tag="den", bufs=1)
        pend = [attn_scores(0, 0), attn_scores(0, 1)]
        prep_block(1)
        resid_block(1)
        # prep for blocks 2,3 spread across chunk-0 ktiles in small pieces
        # so the injected DVE/ACT work never stalls the exp stream for long
        inject = {
            1: lambda: prep_stats((8, 9)),
            2: lambda: prep_stats((10, 11)),
            3: lambda: prep_rsxn(2),
            4: lambda: prep_mm_a(2),
            5: lambda: (prep_mm_b(2), prep_stats((12, 13))),
            6: lambda: prep_stats((14, 15)),
            7: lambda: prep_rsxn(3),
            8: lambda: prep_mm_a(3),
            9: lambda: prep_mm_b(3),
        }
        for kt in range(NKT):
            if kt in inject:
                inject[kt]()
            if kt + 2 < NKT:
                pend.append(attn_scores(0, kt + 2))
            attn_ctxden(0, kt, pend.pop(0))
        nc.vector.reciprocal_approx_fast(den_rec[:, 0, :], den_ps)
        ctx0_ps = ctx_ps

        # chunk 1; chunk-0's tail is injected at kt 2 so its divide /
        # out-projection / residual / DMA overlap chunk-1's attention.
        ctx_ps = ps.tile([128, CHUNK], F32, name="ctx1", tag="ctx", bufs=1)
        den_ps = ps.tile([128, CHUNK], F32, name="den1", tag="den", bufs=1)
        pend = [attn_scores(1, 0), attn_scores(1, 1)]
        for kt in range(NKT):
            if kt == 2:
                chunk_tail(0, ctx0_ps)
            if kt + 2 < NKT:
                pend.append(attn_scores(1, kt + 2))
            attn_ctxden(1, kt, pend.pop(0))
        nc.vector.reciprocal_approx_fast(den_rec[:, 1, :], den_ps)
        chunk_tail(1, ctx_ps)

        pPool.release()
        ps.release()
        sbTmp.release()
        sbBig.release()
        sbW.release()
        consts.release()

    nc.compile()
    return nc


def _get_compiled():
    global _compiled
    if _compiled is None:
        _compiled = _build()
    return _compiled


# device position j <- host row (j%128)*16 + j//128
_DEV2HOST = (np.arange(S) % 128) * NT + np.arange(S) // 128
_HOSTPERM = np.empty(S, dtype=np.int64)
_HOSTPERM[_DEV2HOST] = np.arange(S)


def kernel(x, Wq, bq, Wk, bk, Wv, bv, gamma, beta, Wo, bo):
    x = np.asarray(x, dtype=np.float32)
    vecs = np.stack(
        [np.asarray(a, dtype=np.float32) for a in (gamma, beta, bq, bk, bv, bo)]
    )
    wq = np.ascontiguousarray(np.asarray(Wq, dtype=np.float32))
    wk = np.ascontiguousarray(np.asarray(Wk, dtype=np.float32))
    wv = np.ascontiguousarray(np.asarray(Wv, dtype=np.float32))
    wo = np.ascontiguousarray(np.asarray(Wo, dtype=np.float32))

    nc = _get_compiled()

    in_maps = []
    for c in range(N_CORES):
        b, half = c // 2, c % 2
        off = half * QH
        xroll = np.roll(x[b], -off, axis=0)
        xin = np.ascontiguousarray(xroll[_HOSTPERM].astype(ml_dtypes.bfloat16))
        in_maps.append(
            {"xkv": xin, "wq": wq, "wk": wk, "wv": wv, "wo": wo, "vecs": vecs}
        )

    res = run_bass_kernel_spmd(nc, in_maps, core_ids=list(range(N_CORES)), trace=False)

    out = np.empty((B, S, D), dtype=np.float32)
    for c in range(N_CORES):
        b, half = c // 2, c % 2
        off = half * QH
        out[b, off : off + QH, :] = res.results[c]["outT"].T
    return out
